# revision 81
# baseline (speedup 1.0000x reference)
"""Multi-head attention (B=8, S=1024, D=768, H=12) on 8 TRN2 NeuronCores.

Sharding: pure batch parallelism - one batch element per core, weights
replicated. No collectives needed.

v2: engine-rebalanced pipeline. The key structural change vs v1 is that all
softmax-weight transposes ([q,k] -> [k,q]) and the attn transpose run on the
DMA XBAR (dma_start_transpose, 16x128 tiles) instead of PE+PSUM staging +
ACT evacuation. The XBAR's blocked 3D output (out[p, j, q] = in[q, j*128+p])
is exactly the per-128-chunk transposed layout the AV / projection matmuls
need as stationary operands.

Per-core pipeline (tokens T=1024, D=768, H=12 heads of HD=64):
  1. x loaded and PE-transposed to xT (fp32 -> fp32r via ACT evacuation);
     W_qkv DMA'd fp32 and rounded to fp32r by GPSIMD copies (V columns
     first); W_proj SWDGE-cast to bf16.
  2. V [T, 768] (bf16, with a ones column per head whose AV output column is
     the softmax denominator); Q^T,K^T [768, T] kept fp32r, Q pre-scaled x8
     during ACT evacuation so softmax needs no separate scale pass.
  3. Per head h, per query chunk qi (causal k <= (qi+1)*128):
       s = Q_h K_h^T (fp32r)         [PE]
       diagonal block mask add       [DVE]
       m = rowmax(s), negated        [DVE]
       w = exp(s - m) -> bf16        [ACT]
       wT = XBAR transpose of w      [DMA, issued from SP]
       o|Z = w @ [V_h | 1]           [PE, bf16]
       attn[:, h*64:] = o * (1/Z)    [recip DVE, scale copy ACT]
     QK chunk pairs are software-pipelined with the heads that consume them.
  4. In the last head's sweep each finished token chunk is XBAR-transposed
     and projected: y = attn @ W_proj + b, then DMA'd out.

Measured vs the fp32 jax reference: rel err ~3e-3 (scores and softmax stats
in fp32/tf32; only w/V/attn/W_proj are bf16).
"""

import numpy as np

import concourse.bass as bass
import concourse.mybir as mybir
import concourse.tile as tile
from concourse import bacc
from concourse.bass_utils import run_bass_kernel_spmd
from concourse.masks import make_causal_mask, make_identity

B, S, D = 8, 1024, 768
H, HD = 12, 64
HV = 65  # V block width per head: 64 value cols + the ones column
NT = S // 128   # 8 token chunks
ND = D // 128   # 6 d chunks
F32 = mybir.dt.float32
F32R = mybir.dt.float32r
BF16 = mybir.dt.bfloat16

N_CORES = 8


def bank_chunks(size):
    """Split [0, size) into matmul-N chunks that each sit in one PSUM bank
    (fp32 bank = 512 elems) and are >=256 where possible (fp32r full rate)."""
    out = []
    start = 0
    while start < size:
        end = min(start + 512, size, (start // 512 + 1) * 512)
        out.append((start, end))
        start = end
    return out


def build_mha(nc):
    x_d = nc.dram_tensor("x", [S, D], F32, kind="ExternalInput")
    wqkv_d = nc.dram_tensor("W_qkv", [D, 3 * D], F32, kind="ExternalInput")
    wproj_d = nc.dram_tensor("W_proj", [D, D], F32, kind="ExternalInput")
    bproj_d = nc.dram_tensor("b_proj", [1, D], F32, kind="ExternalInput")
    out_d = nc.dram_tensor("out", [S, D], F32, kind="ExternalOutput")

    with tile.TileContext(nc) as tc:
        with (
            tc.tile_pool(name="persist", bufs=1) as pp,
            tc.tile_pool(name="psum", bufs=1, space="PSUM") as psum,
        ):
            def pring():
                # big ring: score tiles + V/QK/output projections + startup
                # staging (3 x 2 banks)
                return psum.tile([128, 1024], F32, name="p1", tag="pring", bufs=3)

            def po_tile():
                # AV outputs + startup staging ring
                return psum.tile([128, 512], F32, name="pt1", tag="ptiny", bufs=2)

            def pwide():
                return pring()

            # ---- constants ----
            ident_f32 = pp.tile([128, 128], F32, name="ident_f32", tag="ident_f32")
            make_identity(nc, ident_f32[:])
            # zero-padded causal mask, fp32r: the score matmul group for the
            # bank chunk containing the diagonal first writes this into PSUM
            # (start=True) and accumulates QK on top, so masking costs PE
            # stream cycles instead of a DVE pass + cross-engine hop.
            # (fp32r tiles must be written by compute ops - memset/select on
            # them is invalid ISA - so build in f32 and copy.)
            bigmaskZr = pp.tile([128, 512], F32R, name="bigmaskZr", tag="bigmaskZr")
            ident_r = pp.tile([128, 128], F32R, name="ident_r", tag="ident_r")
            nc.vector.tensor_copy(ident_r[:], ident_f32[:])
            ident_bf16 = pp.tile([128, 128], BF16, name="ident_bf16", tag="ident_bf16")
            nc.vector.tensor_copy(ident_bf16[:], ident_f32[:])

            # b_proj broadcast to 128 partitions via K=1 outer product
            b_row = pp.tile([1, D], F32, name="b_row", tag="b_row")
            nc.sync.dma_start(b_row[:], bproj_d[:])
            ones_col = pp.tile([1, 128], F32, name="ones_col", tag="ones_col")
            nc.vector.memset(ones_col[:], 1.0)
            b_bcast = pp.tile([128, D], F32, name="b_bcast", tag="b_bcast")
            pb = po_tile()
            for c0, c1 in bank_chunks(D):
                nc.tensor.matmul(
                    pb[:, 0:c1 - c0], ones_col[:], b_row[:, c0:c1],
                    start=True, stop=True,
                )
                nc.vector.tensor_copy(b_bcast[:, c0:c1], pb[:, 0:c1 - c0])

            # ---- persistent activations ----
            qkT = [pp.tile([128, S], F32R, name=f"qkT{m}", tag=f"qkT{m}") for m in range(12)]
            v_sb = [pp.tile([128, H * HV], BF16, name=f"v{qi}", tag=f"v{qi}") for qi in range(NT)]
            attn = [pp.tile([128, D], BF16, name=f"attn{qi}", tag=f"attn{qi}") for qi in range(NT)]
            wq = [pp.tile([128, 3 * D], F32R, name=f"wq{di}", tag=f"wq{di}") for di in range(ND)]
            wp = [pp.tile([128, D], BF16, name=f"wp{di}", tag=f"wp{di}") for di in range(ND)]

            with (
                tc.tile_pool(name="xpool", bufs=1) as xp,
                tc.tile_pool(name="wqstage", bufs=2) as wqs,
                tc.tile_pool(name="xtp", bufs=1) as xtp,
                tc.tile_pool(name="softmax", bufs=3) as p3s,
                tc.tile_pool(name="wtp", bufs=3) as wtp,
                tc.tile_pool(name="attp", bufs=2) as a3p,
                tc.tile_pool(name="ypool", bufs=2) as yp,
            ):
                # ---- loads. One serialized DMA device moves ~31us of
                # inputs, so order decides when compute can start:
                #   V cols + x tokens 0-3 -> V projection from ~10us
                #   Q/K first halves (cols 0:384 of each part) -> the head-0/1
                #     QK projections run during the V phase
                #   x tokens 4-7, then Q/K second halves.
                # GPSIMD copies do the fp32 -> tf32 rounding. ----
                def wq_load(part, di, eng, round_eng=None):
                    wq_stage = wqs.tile([128, D], F32, name="wq_stage", tag="wq_stage")
                    eng.dma_start(
                        wq_stage[:],
                        wqkv_d[di * 128:(di + 1) * 128,
                               part * D:(part + 1) * D],
                    )
                    dst = wq[di][:, part * D:(part + 1) * D]
                    if round_eng == "act":
                        nc.scalar.copy(dst, wq_stage[:])
                    elif round_eng == "dve":
                        nc.vector.tensor_copy(dst, wq_stage[:])
                    else:
                        nc.gpsimd.tensor_copy(dst, wq_stage[:])

                mz = wqs.tile([128, D], F32, name="wq_stage", tag="wq_stage")
                nc.gpsimd.memset(mz[:, :512], 0.0)
                make_causal_mask(nc, mz[:, 384:512], mask_val=-1e10)
                nc.vector.tensor_copy(bigmaskZr[:], mz[:, :512])

                xT = [xtp.tile([128, S], F32R, name=f"xT{di}", tag=f"xT{di}") for di in range(ND)]

                def x_load(qi):
                    # x load + transpose (fp32 PE transpose, ACT evacuation
                    # rounds to fp32r)
                    x_t = xp.tile([128, D], F32, name="x_t", tag="x_t")
                    nc.sync.dma_start(x_t[:], x_d[qi * 128:(qi + 1) * 128, :])
                    for di0 in range(0, ND, 4):
                        nb = min(4, ND - di0)
                        pt = po_tile()
                        for j in range(nb):
                            nc.tensor.transpose(
                                pt[:, j * 128:(j + 1) * 128],
                                x_t[:, (di0 + j) * 128:(di0 + j + 1) * 128],
                                ident_f32[:],
                            )
                        for j in range(nb):
                            nc.vector.tensor_copy(
                                xT[di0 + j][:, qi * 128:(qi + 1) * 128],
                                pt[:, j * 128:(j + 1) * 128],
                            )

                for di in range(ND):
                    wq_load(2, di, nc.scalar)
                for qi in range(NT):
                    x_load(qi)
                # Q/K rounding on ACT: the Pool engine's ~1.4us/copy
                # throughput otherwise gates when the K weights are ready
                for di in range(ND):
                    wq_load(0, di, nc.sync)
                for di in range(ND):
                    # K rounds on DVE: it is the last-needed input and the
                    # Pool engine's copy throughput would gate head 0
                    wq_load(1, di, nc.sync, round_eng="dve")

                # QK projection work is enqueued per 512-token half-chunk as
                # single-instruction fillers and trickled out between
                # attention blocks (and during the V phase): a 12-matmul
                # burst in the in-order PE stream otherwise stalls the
                # softmax pipeline ~4us per injection.
                filler = []
                filler_done = {}

                def qk_chunk(m):
                    # two independent 512-token halves, each in a 1-bank slot
                    # of the AV-output ring: a 2-bank pqk tile in the score
                    # ring would be held ~14 block-slots at the 1-op/block
                    # trickle rate and starve the score pipeline
                    filler_done[m] = False
                    for h0, h1 in ((0, 512), (512, 1024)):
                        pqk = pwide()
                        for di in range(ND):
                            def mm(di=di, pqk=pqk, m=m, h0=h0, h1=h1):
                                nc.tensor.matmul(
                                    pqk[:, :512],
                                    wq[di][:, m * 128:(m + 1) * 128],
                                    xT[di][:, h0:h1],
                                    start=(di == 0), stop=(di == ND - 1),
                                )
                            filler.append(mm)

                        def ev(pqk=pqk, m=m, h0=h0, h1=h1):
                            if m < 6:
                                # pre-scale Q by 8: scores come out as 8*s and
                                # the softmax needs no separate x8 pass
                                nc.vector.tensor_scalar_mul(
                                    qkT[m][:, h0:h1], pqk[:, :512], 8.0)
                            else:
                                nc.vector.tensor_copy(
                                    qkT[m][:, h0:h1], pqk[:, :512])
                            if h1 == 1024:
                                filler_done[m] = True
                        filler.append(ev)

                def pop_filler(n):
                    for _ in range(min(n, len(filler))):
                        filler.pop(0)()

                def need_qkT(m):
                    while not filler_done.get(m, True):
                        pop_filler(1)

                # head-0's Q chunk is popped during the V phase (its weights
                # land mid-phase); the K chunk right after
                qk_chunk(0)

                # ---- V in [token, dv] layout, with ones column per head ----
                for qi in range(NT):
                    pv = pwide()
                    for c0, c1 in bank_chunks(D):
                        for di in range(ND):
                            nc.tensor.matmul(
                                pv[:, c0:c1],
                                xT[di][:, qi * 128:(qi + 1) * 128],
                                wq[di][:, 2 * D + c0:2 * D + c1],
                                start=(di == 0), stop=(di == ND - 1),
                            )
                    nc.gpsimd.memset(
                        v_sb[qi][:].rearrange("p (h v) -> p h v", v=HV)[:, :, HD:], 1.0
                    )
                    nc.vector.tensor_copy(
                        v_sb[qi][:].rearrange("p (h v) -> p h v", v=HV)[:, :, :HD],
                        pv[:, :D].rearrange("p (h v) -> p h v", v=HD),
                    )
                    if qi >= 5:
                        pop_filler(5)

                qk_chunk(6)

                for di in range(ND):
                    # SWDGE cast fp32 -> bf16 during load; issued late so the
                    # Pool engine rounds the W_qkv tiles first
                    nc.gpsimd.dma_start(
                        wp[di][:], wproj_d[di * 128:(di + 1) * 128, :]
                    )

                def attn_front(h, qi):
                    """scores -> mask -> rowmax -> exp -> XBAR transpose
                    launch. Returns the in-flight wT tile for attn_back."""
                    qoff = (h % 2) * 64
                    Qt = qkT[h // 2]
                    Kt = qkT[6 + h // 2]
                    ks = (qi + 1) * 128
                    lhs = Qt[qoff:qoff + 64, qi * 128:(qi + 1) * 128]
                    ps = pring()
                    chunks = bank_chunks(ks)
                    for c0, c1 in chunks[:-1]:
                        nc.tensor.matmul(
                            ps[:, c0:c1],
                            lhs,
                            Kt[qoff:qoff + 64, c0:c1],
                            start=True, stop=True,
                        )
                    c0, c1 = chunks[-1]
                    nc.tensor.matmul(
                        ps[:, c0:c1],
                        ident_r[:],
                        bigmaskZr[:, 512 - (c1 - c0):],
                        start=True, stop=False,
                    )
                    nc.tensor.matmul(
                        ps[:, c0:c1],
                        lhs,
                        Kt[qoff:qoff + 64, c0:c1],
                        start=False, stop=True,
                    )
                    neg8m = p3s.tile([128, 1], F32, name="neg8m", tag="neg8m", bufs=8)
                    nc.vector.reduce_max(
                        out=neg8m[:], in_=ps[:, :ks],
                        axis=mybir.AxisListType.X, negate=True,
                    )
                    w_t = p3s.tile([128, S], BF16, name="w_t", tag="w_t", bufs=4)
                    nc.scalar.activation(
                        w_t[:, :ks], ps[:, :ks],
                        mybir.ActivationFunctionType.Exp,
                        bias=neg8m[:], scale=1.0,
                    )
                    # w[q, k] -> wT[k in chunk, ki, q] on the DMA XBAR;
                    # blocked output: wT[p, ki, q] = w[q, ki*128+p].
                    # Per-qi tags: the same slot is reused only 8 blocks
                    # later (by the next head), a deeper effective ring than
                    # fixed-size buffers could afford in SBUF.
                    wT = wtp.tile([128, qi + 1, 128], BF16, name="wT",
                                  tag=f"wT{qi}", bufs=1)
                    nc.sync.dma_start(wT[:], w_t[:, :ks], transpose=True)
                    return wT

                def attn_av(h, qi, wT):
                    """AV matmul off the landed transpose."""
                    # o = w @ [V_h | 1]; last column = softmax denominator
                    po = po_tile()
                    for ki in range(qi + 1):
                        nc.tensor.matmul(
                            po[:, :HV],
                            wT[:, ki, :],
                            v_sb[ki][:, h * HV:(h + 1) * HV],
                            start=(ki == 0), stop=(ki == qi),
                        )
                    return po

                def attn_fin(h, qi, po):
                    """normalize: runs two blocks after its AV so the DVE/ACT
                    queue heads never wait on a fresh PE result."""
                    recip = p3s.tile([128, 1], F32, name="recip", tag="recip", bufs=8)
                    nc.vector.reciprocal(recip[:], po[:, HD:HV])
                    nc.scalar.mul(
                        attn[qi][:, h * HD:(h + 1) * HD],
                        po[:, :HD],
                        recip[:],
                    )

                def proj_front(qi):
                    """all heads done for token chunk qi: PE-transpose it.
                    (PE, not the XBAR: these run at the kernel tail where PE
                    is idle anyway -- the XBAR's ~2.4us latency would gate
                    the final projections and let the PE p-state drop.)"""
                    pa = pring().bitcast(BF16)
                    for di in range(ND):
                        nc.tensor.transpose(
                            pa[:, di * 128:(di + 1) * 128],
                            attn[qi][:, di * 128:(di + 1) * 128],
                            ident_bf16[:],
                        )
                    att3 = a3p.tile([128, ND, 128], BF16, name="att3", tag="att3", bufs=2)
                    nc.vector.tensor_copy(
                        att3[:].rearrange("p a b -> p (a b)"), pa[:, :ND * 128]
                    )
                    return att3

                def proj_back(qi, att3):
                    y_t = yp.tile([128, D], F32, name="y_t", tag="y_t")
                    py = pwide()
                    for c0, c1 in bank_chunks(D):
                        for di in range(ND):
                            nc.tensor.matmul(
                                py[:, c0:c1],
                                att3[:, di, :],
                                wp[di][:, c0:c1],
                                start=(di == 0), stop=(di == ND - 1),
                            )
                    nc.vector.tensor_tensor(
                        out=y_t[:], in0=py[:, :D], in1=b_bcast[:],
                        op=mybir.AluOpType.add,
                    )
                    nc.scalar.dma_start(
                        out_d[qi * 128:(qi + 1) * 128, :], y_t[:]
                    )

                # software pipeline: each block's AV runs LAG_AV blocks after
                # its XBAR transpose was launched (hiding the ~2.5us DMA
                # latency behind other blocks' scores/exp), and its normalize
                # a further LAG_FIN-LAG_AV blocks later (so DVE/ACT queue
                # heads never wait on fresh PE results). The last head's
                # finished token chunks go through the same lagging for their
                # projection. QK chunk pairs for the next head pair are issued
                # mid-head to spread PSUM-ring pressure.
                LAG_AV = 4
                LAG_FIN = 4
                front_q = []
                av_q = []
                proj_q = []

                def drain(front_limit, av_limit):
                    while len(front_q) > front_limit:
                        h, qi, wT = front_q.pop(0)
                        av_q.append((h, qi, attn_av(h, qi, wT)))
                    while len(av_q) > av_limit:
                        h, qi, po = av_q.pop(0)
                        attn_fin(h, qi, po)
                        if h == PROJ_HEAD:
                            proj_q.append((qi, proj_front(qi)))
                        if len(proj_q) > 2:
                            proj_back(*proj_q.pop(0))

                pop_filler(len(filler))
                # the last head pair is interleaved (11,qi),(10,qi) so each
                # token chunk finishes all 12 heads mid-pipeline and its
                # projection spreads out instead of bunching at the end
                PROJ_HEAD = 10
                blocks = [(h, qi) for h in range(10) for qi in range(NT)]
                for qi in range(NT):
                    blocks.append((11, qi))
                    blocks.append((10, qi))
                seen_h = set()
                for h, qi in blocks:
                    r = h // 2
                    if h not in seen_h:
                        seen_h.add(h)
                        need_qkT(r)
                        need_qkT(6 + r)
                    front_q.append((h, qi, attn_front(h, qi)))
                    drain(LAG_AV, LAG_FIN - LAG_AV)
                    pop_filler(1)
                    if r < 5:
                        if h % 2 == 0 and qi == 7:
                            qk_chunk(r + 1)
                        elif h % 2 == 1 and qi == 3:
                            qk_chunk(7 + r)
                drain(0, 0)
                while proj_q:
                    proj_back(*proj_q.pop(0))

    nc.compile()
    return nc


_NC_CACHE = None


def _get_nc():
    global _NC_CACHE
    if _NC_CACHE is None:
        nc = bacc.Bacc(
            "TRN2",
            target_bir_lowering=False,
            debug=False,
            num_devices=N_CORES,
        )
        build_mha(nc)
        _NC_CACHE = nc
    return _NC_CACHE


def kernel(x, W_qkv, W_proj, b_proj):
    nc = _get_nc()
    x = np.ascontiguousarray(np.asarray(x, dtype=np.float32))
    W_qkv = np.ascontiguousarray(np.asarray(W_qkv, dtype=np.float32))
    W_proj = np.ascontiguousarray(np.asarray(W_proj, dtype=np.float32))
    b_proj = np.ascontiguousarray(
        np.asarray(b_proj, dtype=np.float32).reshape(1, D)
    )
    in_maps = [
        {"x": x[b], "W_qkv": W_qkv, "W_proj": W_proj, "b_proj": b_proj}
        for b in range(N_CORES)
    ]
    res = run_bass_kernel_spmd(nc, in_maps, core_ids=list(range(N_CORES)))
    return np.stack([res.results[b]["out"] for b in range(N_CORES)], axis=0)


# revision 82
# speedup vs baseline: 1.0020x; 1.0020x over previous
"""Multi-head attention (B=8, S=1024, D=768, H=12) on 8 TRN2 NeuronCores.

Sharding: pure batch parallelism - one batch element per core, weights
replicated. No collectives needed.

v2: engine-rebalanced pipeline. The key structural change vs v1 is that all
softmax-weight transposes ([q,k] -> [k,q]) and the attn transpose run on the
DMA XBAR (dma_start_transpose, 16x128 tiles) instead of PE+PSUM staging +
ACT evacuation. The XBAR's blocked 3D output (out[p, j, q] = in[q, j*128+p])
is exactly the per-128-chunk transposed layout the AV / projection matmuls
need as stationary operands.

Per-core pipeline (tokens T=1024, D=768, H=12 heads of HD=64):
  1. x loaded and PE-transposed to xT (fp32 -> fp32r via ACT evacuation);
     W_qkv DMA'd fp32 and rounded to fp32r by GPSIMD copies (V columns
     first); W_proj SWDGE-cast to bf16.
  2. V [T, 768] (bf16, with a ones column per head whose AV output column is
     the softmax denominator); Q^T,K^T [768, T] kept fp32r, Q pre-scaled x8
     during ACT evacuation so softmax needs no separate scale pass.
  3. Per head h, per query chunk qi (causal k <= (qi+1)*128):
       s = Q_h K_h^T (fp32r)         [PE]
       diagonal block mask add       [DVE]
       m = rowmax(s), negated        [DVE]
       w = exp(s - m) -> bf16        [ACT]
       wT = XBAR transpose of w      [DMA, issued from SP]
       o|Z = w @ [V_h | 1]           [PE, bf16]
       attn[:, h*64:] = o * (1/Z)    [recip DVE, scale copy ACT]
     QK chunk pairs are software-pipelined with the heads that consume them.
  4. In the last head's sweep each finished token chunk is XBAR-transposed
     and projected: y = attn @ W_proj + b, then DMA'd out.

Measured vs the fp32 jax reference: rel err ~3e-3 (scores and softmax stats
in fp32/tf32; only w/V/attn/W_proj are bf16).
"""

import numpy as np

import concourse.bass as bass
import concourse.mybir as mybir
import concourse.tile as tile
from concourse import bacc
from concourse.bass_utils import run_bass_kernel_spmd
from concourse.masks import make_causal_mask, make_identity

B, S, D = 8, 1024, 768
H, HD = 12, 64
HV = 65  # V block width per head: 64 value cols + the ones column
NT = S // 128   # 8 token chunks
ND = D // 128   # 6 d chunks
F32 = mybir.dt.float32
F32R = mybir.dt.float32r
BF16 = mybir.dt.bfloat16

N_CORES = 8


def bank_chunks(size):
    """Split [0, size) into matmul-N chunks that each sit in one PSUM bank
    (fp32 bank = 512 elems) and are >=256 where possible (fp32r full rate)."""
    out = []
    start = 0
    while start < size:
        end = min(start + 512, size, (start // 512 + 1) * 512)
        out.append((start, end))
        start = end
    return out


def build_mha(nc):
    x_d = nc.dram_tensor("x", [S, D], F32, kind="ExternalInput")
    wqkv_d = nc.dram_tensor("W_qkv", [D, 3 * D], F32, kind="ExternalInput")
    wproj_d = nc.dram_tensor("W_proj", [D, D], F32, kind="ExternalInput")
    bproj_d = nc.dram_tensor("b_proj", [1, D], F32, kind="ExternalInput")
    out_d = nc.dram_tensor("out", [S, D], F32, kind="ExternalOutput")

    with tile.TileContext(nc) as tc:
        with (
            tc.tile_pool(name="persist", bufs=1) as pp,
            tc.tile_pool(name="psum", bufs=1, space="PSUM") as psum,
        ):
            def pring():
                # big ring: score tiles + V/QK/output projections + startup
                # staging (3 x 2 banks)
                return psum.tile([128, 1024], F32, name="p1", tag="pring", bufs=3)

            def po_tile():
                # AV outputs + startup staging ring
                return psum.tile([128, 512], F32, name="pt1", tag="ptiny", bufs=2)

            def pwide():
                return pring()

            # ---- constants ----
            ident_f32 = pp.tile([128, 128], F32, name="ident_f32", tag="ident_f32")
            make_identity(nc, ident_f32[:])
            # zero-padded causal mask, fp32r: the score matmul group for the
            # bank chunk containing the diagonal first writes this into PSUM
            # (start=True) and accumulates QK on top, so masking costs PE
            # stream cycles instead of a DVE pass + cross-engine hop.
            # (fp32r tiles must be written by compute ops - memset/select on
            # them is invalid ISA - so build in f32 and copy.)
            bigmaskZr = pp.tile([128, 512], F32R, name="bigmaskZr", tag="bigmaskZr")
            ident_r = pp.tile([128, 128], F32R, name="ident_r", tag="ident_r")
            nc.vector.tensor_copy(ident_r[:], ident_f32[:])
            ident_bf16 = pp.tile([128, 128], BF16, name="ident_bf16", tag="ident_bf16")
            nc.vector.tensor_copy(ident_bf16[:], ident_f32[:])

            # b_proj broadcast to 128 partitions via K=1 outer product
            b_row = pp.tile([1, D], F32, name="b_row", tag="b_row")
            nc.sync.dma_start(b_row[:], bproj_d[:])
            ones_col = pp.tile([1, 128], F32, name="ones_col", tag="ones_col")
            nc.vector.memset(ones_col[:], 1.0)
            b_bcast = pp.tile([128, D], F32, name="b_bcast", tag="b_bcast")
            pb = po_tile()
            for c0, c1 in bank_chunks(D):
                nc.tensor.matmul(
                    pb[:, 0:c1 - c0], ones_col[:], b_row[:, c0:c1],
                    start=True, stop=True,
                )
                nc.vector.tensor_copy(b_bcast[:, c0:c1], pb[:, 0:c1 - c0])

            # ---- persistent activations ----
            qkT = [pp.tile([128, S], F32R, name=f"qkT{m}", tag=f"qkT{m}") for m in range(12)]
            v_sb = [pp.tile([128, H * HV], BF16, name=f"v{qi}", tag=f"v{qi}") for qi in range(NT)]
            attn = [pp.tile([128, D], BF16, name=f"attn{qi}", tag=f"attn{qi}") for qi in range(NT)]
            wq = [pp.tile([128, 3 * D], F32R, name=f"wq{di}", tag=f"wq{di}") for di in range(ND)]
            wp = [pp.tile([128, D], BF16, name=f"wp{di}", tag=f"wp{di}") for di in range(ND)]

            with (
                tc.tile_pool(name="xpool", bufs=1) as xp,
                tc.tile_pool(name="wqstage", bufs=2) as wqs,
                tc.tile_pool(name="xtp", bufs=1) as xtp,
                tc.tile_pool(name="softmax", bufs=3) as p3s,
                tc.tile_pool(name="wtp", bufs=3) as wtp,
                tc.tile_pool(name="attp", bufs=2) as a3p,
                tc.tile_pool(name="ypool", bufs=2) as yp,
            ):
                # ---- loads. One serialized DMA device moves ~31us of
                # inputs, so order decides when compute can start:
                #   V cols + x tokens 0-3 -> V projection from ~10us
                #   Q/K first halves (cols 0:384 of each part) -> the head-0/1
                #     QK projections run during the V phase
                #   x tokens 4-7, then Q/K second halves.
                # GPSIMD copies do the fp32 -> tf32 rounding. ----
                def wq_load(part, di, eng, round_eng=None):
                    wq_stage = wqs.tile([128, D], F32, name="wq_stage", tag="wq_stage")
                    eng.dma_start(
                        wq_stage[:],
                        wqkv_d[di * 128:(di + 1) * 128,
                               part * D:(part + 1) * D],
                    )
                    dst = wq[di][:, part * D:(part + 1) * D]
                    if round_eng == "act":
                        nc.scalar.copy(dst, wq_stage[:])
                    elif round_eng == "dve":
                        nc.vector.tensor_copy(dst, wq_stage[:])
                    else:
                        nc.gpsimd.tensor_copy(dst, wq_stage[:])

                mz = wqs.tile([128, D], F32, name="wq_stage", tag="wq_stage")
                nc.gpsimd.memset(mz[:, :512], 0.0)
                make_causal_mask(nc, mz[:, 384:512], mask_val=-1e10)
                nc.vector.tensor_copy(bigmaskZr[:], mz[:, :512])

                xT = [xtp.tile([128, S], F32R, name=f"xT{di}", tag=f"xT{di}") for di in range(ND)]

                def x_load(qi):
                    # x load + transpose (fp32 PE transpose, ACT evacuation
                    # rounds to fp32r)
                    x_t = xp.tile([128, D], F32, name="x_t", tag="x_t")
                    nc.sync.dma_start(x_t[:], x_d[qi * 128:(qi + 1) * 128, :])
                    for di0 in range(0, ND, 4):
                        nb = min(4, ND - di0)
                        pt = po_tile()
                        for j in range(nb):
                            nc.tensor.transpose(
                                pt[:, j * 128:(j + 1) * 128],
                                x_t[:, (di0 + j) * 128:(di0 + j + 1) * 128],
                                ident_f32[:],
                            )
                        for j in range(nb):
                            nc.scalar.copy(
                                xT[di0 + j][:, qi * 128:(qi + 1) * 128],
                                pt[:, j * 128:(j + 1) * 128],
                            )

                for di in range(ND):
                    wq_load(2, di, nc.scalar)
                for qi in range(NT):
                    x_load(qi)
                # Q/K rounding on ACT: the Pool engine's ~1.4us/copy
                # throughput otherwise gates when the K weights are ready
                for di in range(ND):
                    wq_load(0, di, nc.sync)
                for di in range(ND):
                    # K rounds on DVE: it is the last-needed input and the
                    # Pool engine's copy throughput would gate head 0
                    wq_load(1, di, nc.sync, round_eng="dve")

                # QK projection work is enqueued per 512-token half-chunk as
                # single-instruction fillers and trickled out between
                # attention blocks (and during the V phase): a 12-matmul
                # burst in the in-order PE stream otherwise stalls the
                # softmax pipeline ~4us per injection.
                filler = []
                filler_done = {}

                def qk_chunk(m):
                    # two independent 512-token halves, each in a 1-bank slot
                    # of the AV-output ring: a 2-bank pqk tile in the score
                    # ring would be held ~14 block-slots at the 1-op/block
                    # trickle rate and starve the score pipeline
                    filler_done[m] = False
                    for h0, h1 in ((0, 512), (512, 1024)):
                        pqk = pwide()
                        for di in range(ND):
                            def mm(di=di, pqk=pqk, m=m, h0=h0, h1=h1):
                                nc.tensor.matmul(
                                    pqk[:, :512],
                                    wq[di][:, m * 128:(m + 1) * 128],
                                    xT[di][:, h0:h1],
                                    start=(di == 0), stop=(di == ND - 1),
                                )
                            filler.append(mm)

                        def ev(pqk=pqk, m=m, h0=h0, h1=h1):
                            if m < 6:
                                # pre-scale Q by 8: scores come out as 8*s and
                                # the softmax needs no separate x8 pass
                                nc.vector.tensor_scalar_mul(
                                    qkT[m][:, h0:h1], pqk[:, :512], 8.0)
                            else:
                                nc.vector.tensor_copy(
                                    qkT[m][:, h0:h1], pqk[:, :512])
                            if h1 == 1024:
                                filler_done[m] = True
                        filler.append(ev)

                def pop_filler(n):
                    for _ in range(min(n, len(filler))):
                        filler.pop(0)()

                def need_qkT(m):
                    while not filler_done.get(m, True):
                        pop_filler(1)

                # head-0's Q chunk is popped during the V phase (its weights
                # land mid-phase); the K chunk right after
                qk_chunk(0)

                # ---- V in [token, dv] layout, with ones column per head ----
                for qi in range(NT):
                    pv = pwide()
                    for c0, c1 in bank_chunks(D):
                        for di in range(ND):
                            nc.tensor.matmul(
                                pv[:, c0:c1],
                                xT[di][:, qi * 128:(qi + 1) * 128],
                                wq[di][:, 2 * D + c0:2 * D + c1],
                                start=(di == 0), stop=(di == ND - 1),
                            )
                    nc.gpsimd.memset(
                        v_sb[qi][:].rearrange("p (h v) -> p h v", v=HV)[:, :, HD:], 1.0
                    )
                    nc.vector.tensor_copy(
                        v_sb[qi][:].rearrange("p (h v) -> p h v", v=HV)[:, :, :HD],
                        pv[:, :D].rearrange("p (h v) -> p h v", v=HD),
                    )
                    if qi >= 5:
                        pop_filler(5)

                qk_chunk(6)

                for di in range(ND):
                    # SWDGE cast fp32 -> bf16 during load; issued late so the
                    # Pool engine rounds the W_qkv tiles first
                    nc.gpsimd.dma_start(
                        wp[di][:], wproj_d[di * 128:(di + 1) * 128, :]
                    )

                def attn_front(h, qi):
                    """scores -> mask -> rowmax -> exp -> XBAR transpose
                    launch. Returns the in-flight wT tile for attn_back."""
                    qoff = (h % 2) * 64
                    Qt = qkT[h // 2]
                    Kt = qkT[6 + h // 2]
                    ks = (qi + 1) * 128
                    lhs = Qt[qoff:qoff + 64, qi * 128:(qi + 1) * 128]
                    ps = pring()
                    chunks = bank_chunks(ks)
                    for c0, c1 in chunks[:-1]:
                        nc.tensor.matmul(
                            ps[:, c0:c1],
                            lhs,
                            Kt[qoff:qoff + 64, c0:c1],
                            start=True, stop=True,
                        )
                    c0, c1 = chunks[-1]
                    nc.tensor.matmul(
                        ps[:, c0:c1],
                        ident_r[:],
                        bigmaskZr[:, 512 - (c1 - c0):],
                        start=True, stop=False,
                    )
                    nc.tensor.matmul(
                        ps[:, c0:c1],
                        lhs,
                        Kt[qoff:qoff + 64, c0:c1],
                        start=False, stop=True,
                    )
                    neg8m = p3s.tile([128, 1], F32, name="neg8m", tag="neg8m", bufs=8)
                    nc.vector.reduce_max(
                        out=neg8m[:], in_=ps[:, :ks],
                        axis=mybir.AxisListType.X, negate=True,
                    )
                    w_t = p3s.tile([128, S], BF16, name="w_t", tag="w_t", bufs=4)
                    nc.scalar.activation(
                        w_t[:, :ks], ps[:, :ks],
                        mybir.ActivationFunctionType.Exp,
                        bias=neg8m[:], scale=1.0,
                    )
                    # w[q, k] -> wT[k in chunk, ki, q] on the DMA XBAR;
                    # blocked output: wT[p, ki, q] = w[q, ki*128+p].
                    # Per-qi tags: the same slot is reused only 8 blocks
                    # later (by the next head), a deeper effective ring than
                    # fixed-size buffers could afford in SBUF.
                    wT = wtp.tile([128, qi + 1, 128], BF16, name="wT",
                                  tag=f"wT{qi}", bufs=1)
                    nc.sync.dma_start(wT[:], w_t[:, :ks], transpose=True)
                    return wT

                def attn_av(h, qi, wT):
                    """AV matmul off the landed transpose."""
                    # o = w @ [V_h | 1]; last column = softmax denominator
                    po = po_tile()
                    for ki in range(qi + 1):
                        nc.tensor.matmul(
                            po[:, :HV],
                            wT[:, ki, :],
                            v_sb[ki][:, h * HV:(h + 1) * HV],
                            start=(ki == 0), stop=(ki == qi),
                        )
                    return po

                def attn_fin(h, qi, po):
                    """normalize: runs two blocks after its AV so the DVE/ACT
                    queue heads never wait on a fresh PE result."""
                    recip = p3s.tile([128, 1], F32, name="recip", tag="recip", bufs=8)
                    nc.vector.reciprocal(recip[:], po[:, HD:HV])
                    nc.scalar.mul(
                        attn[qi][:, h * HD:(h + 1) * HD],
                        po[:, :HD],
                        recip[:],
                    )

                def proj_front(qi):
                    """all heads done for token chunk qi: PE-transpose it.
                    (PE, not the XBAR: these run at the kernel tail where PE
                    is idle anyway -- the XBAR's ~2.4us latency would gate
                    the final projections and let the PE p-state drop.)"""
                    pa = pring().bitcast(BF16)
                    for di in range(ND):
                        nc.tensor.transpose(
                            pa[:, di * 128:(di + 1) * 128],
                            attn[qi][:, di * 128:(di + 1) * 128],
                            ident_bf16[:],
                        )
                    att3 = a3p.tile([128, ND, 128], BF16, name="att3", tag="att3", bufs=2)
                    nc.vector.tensor_copy(
                        att3[:].rearrange("p a b -> p (a b)"), pa[:, :ND * 128]
                    )
                    return att3

                def proj_back(qi, att3):
                    y_t = yp.tile([128, D], F32, name="y_t", tag="y_t")
                    py = pwide()
                    for c0, c1 in bank_chunks(D):
                        for di in range(ND):
                            nc.tensor.matmul(
                                py[:, c0:c1],
                                att3[:, di, :],
                                wp[di][:, c0:c1],
                                start=(di == 0), stop=(di == ND - 1),
                            )
                    nc.vector.tensor_tensor(
                        out=y_t[:], in0=py[:, :D], in1=b_bcast[:],
                        op=mybir.AluOpType.add,
                    )
                    nc.scalar.dma_start(
                        out_d[qi * 128:(qi + 1) * 128, :], y_t[:]
                    )

                # software pipeline: each block's AV runs LAG_AV blocks after
                # its XBAR transpose was launched (hiding the ~2.5us DMA
                # latency behind other blocks' scores/exp), and its normalize
                # a further LAG_FIN-LAG_AV blocks later (so DVE/ACT queue
                # heads never wait on fresh PE results). The last head's
                # finished token chunks go through the same lagging for their
                # projection. QK chunk pairs for the next head pair are issued
                # mid-head to spread PSUM-ring pressure.
                LAG_AV = 4
                LAG_FIN = 4
                front_q = []
                av_q = []
                proj_q = []

                def drain(front_limit, av_limit):
                    while len(front_q) > front_limit:
                        h, qi, wT = front_q.pop(0)
                        av_q.append((h, qi, attn_av(h, qi, wT)))
                    while len(av_q) > av_limit:
                        h, qi, po = av_q.pop(0)
                        attn_fin(h, qi, po)
                        if h == PROJ_HEAD:
                            proj_q.append((qi, proj_front(qi)))
                        if len(proj_q) > 2:
                            proj_back(*proj_q.pop(0))

                pop_filler(len(filler))
                # the last head pair is interleaved (11,qi),(10,qi) so each
                # token chunk finishes all 12 heads mid-pipeline and its
                # projection spreads out instead of bunching at the end
                PROJ_HEAD = 10
                blocks = [(h, qi) for h in range(10) for qi in range(NT)]
                for qi in range(NT):
                    blocks.append((11, qi))
                    blocks.append((10, qi))
                seen_h = set()
                for h, qi in blocks:
                    r = h // 2
                    if h not in seen_h:
                        seen_h.add(h)
                        need_qkT(r)
                        need_qkT(6 + r)
                    front_q.append((h, qi, attn_front(h, qi)))
                    drain(LAG_AV, LAG_FIN - LAG_AV)
                    pop_filler(1)
                    if r < 5:
                        if h % 2 == 0 and qi == 7:
                            qk_chunk(r + 1)
                        elif h % 2 == 1 and qi == 3:
                            qk_chunk(7 + r)
                drain(0, 0)
                while proj_q:
                    proj_back(*proj_q.pop(0))

    nc.compile()
    return nc


_NC_CACHE = None


def _get_nc():
    global _NC_CACHE
    if _NC_CACHE is None:
        nc = bacc.Bacc(
            "TRN2",
            target_bir_lowering=False,
            debug=False,
            num_devices=N_CORES,
        )
        build_mha(nc)
        _NC_CACHE = nc
    return _NC_CACHE


def kernel(x, W_qkv, W_proj, b_proj):
    nc = _get_nc()
    x = np.ascontiguousarray(np.asarray(x, dtype=np.float32))
    W_qkv = np.ascontiguousarray(np.asarray(W_qkv, dtype=np.float32))
    W_proj = np.ascontiguousarray(np.asarray(W_proj, dtype=np.float32))
    b_proj = np.ascontiguousarray(
        np.asarray(b_proj, dtype=np.float32).reshape(1, D)
    )
    in_maps = [
        {"x": x[b], "W_qkv": W_qkv, "W_proj": W_proj, "b_proj": b_proj}
        for b in range(N_CORES)
    ]
    res = run_bass_kernel_spmd(nc, in_maps, core_ids=list(range(N_CORES)))
    return np.stack([res.results[b]["out"] for b in range(N_CORES)], axis=0)
